# revision 4
# baseline (speedup 1.0000x reference)
"""Causal self-attention (B=4, T=2048, C=1024, H=16, hs=64) on 8 trn2 cores.

Sharding: core c = batch (c//2) x head-group (c%2, 8 heads each).
Each core computes, for its (batch, 8 heads):
  - QKV projection against its slice of w_attn (transposed layouts on chip),
  - causal softmax attention (flash-style, no max subtraction -- scores are
    O(1) for this problem so exp is numerically safe; softmax denominator
    comes for free as a 65th "ones" row appended to V in the PV matmul),
  - partial output projection against its 512 rows of w_o, transposed.
Host side: per-batch pair partials are summed (the tensor-parallel
all-reduce done at unshard time) and transposed back.
"""

import numpy as np

import concourse.bass as bass
import concourse.mybir as mybir
import concourse.tile as tile

N_CORES = 8
B, T, C = 4, 2048, 1024
H_PER_CORE = 8
HS = 64
P = 128
KCH = C // P  # 8 contraction chunks for the projections
NCH = T // 512  # 4 token chunks of 512
QT = T // 512  # 4 query tiles of 512
NEG = -1.0e5
SCALE = 1.0 / np.sqrt(HS)

F32 = mybir.dt.float32


def legalize_sync_waits(nc, max_waits=1):
    """Split multi-sem-wait instructions into chains of single-wait nops.

    The installed walrus codegen rejects instructions whose sync_info
    carries more than one wait. Same-engine program order makes moving the
    extra waits onto preceding nops semantically identical.
    """
    for f in nc.m.functions:
        for b in f.blocks:
            new_insts = []
            for inst in b.instructions:
                si = inst.sync_info
                if si and si.on_wait and len(si.on_wait) > max_waits:
                    waits = list(si.on_wait)
                    extra, keep = waits[:-max_waits], waits[-max_waits:]
                    for i in range(0, len(extra), max_waits):
                        chunk = extra[i : i + max_waits]
                        nop = mybir.InstNoOp(
                            name=f"{inst.name}-ws{i}",
                            engine=inst.engine,
                            ins=[],
                            outs=[],
                            sync_info=mybir.SyncInfo(on_wait=chunk, on_update=[]),
                        )
                        new_insts.append(nop)
                    inst.sync_info = mybir.SyncInfo(
                        on_wait=keep, on_update=list(si.on_update or [])
                    )
                new_insts.append(inst)
            b.instructions = new_insts


def build_attention_body(nc, tc, ctx, tensors):
    """Emit one iteration of the per-core attention computation."""
    x_t, w_qk, w_v, w_o, mask, out_t = tensors
    Exp = mybir.ActivationFunctionType.Exp

    x_t3 = x_t[:].rearrange("(ko ki) t -> ki ko t", ki=P)  # [128, 8, 2048]
    w_qk3 = w_qk[:].rearrange("(ko ki) m -> ki ko m", ki=P)  # [128, 8, 1024]
    w_v3 = w_v[:].rearrange("(ko ki) m -> ki ko m", ki=P)  # [128, 8, 512]
    w_o3 = w_o[:].rearrange("(ko ki) m -> ki ko m", ki=P)  # [128, 4, 1024]
    out2 = out_t[:]  # [1024, 2048]

    consts = ctx.enter_context(tc.tile_pool(name="consts", bufs=1))
    big = ctx.enter_context(tc.tile_pool(name="big", bufs=1))
    xin = ctx.enter_context(tc.tile_pool(name="xin", bufs=1))
    wqk_p = ctx.enter_context(tc.tile_pool(name="wqk_p", bufs=2))
    expp = ctx.enter_context(tc.tile_pool(name="expp", bufs=3))
    smallp = ctx.enter_context(tc.tile_pool(name="smallp", bufs=2))
    dramp = ctx.enter_context(tc.tile_pool(name="dramp", bufs=2, space="DRAM"))
    outsb = ctx.enter_context(tc.tile_pool(name="outsb", bufs=2))
    psum = ctx.enter_context(tc.tile_pool(name="psum", bufs=2, space="PSUM"))

    mask_sb = consts.tile([P, P], F32, name="mask_sb")
    nc.sync.dma_start(out=mask_sb, in_=mask[:])
    w_v_sb = consts.tile([P, KCH, 512], F32, name="w_v_sb")
    nc.sync.dma_start(out=w_v_sb, in_=w_v3)
    w_o_sb = consts.tile([P, 4, 1024], F32, name="w_o_sb")
    nc.sync.dma_start(out=w_o_sb, in_=w_o3)

    # Per head-pair resident Q^T / K^T ([2*64 rows, T]) and Y^T tiles.
    q_sb = [big.tile([P, T], F32, name=f"q_sb{p}") for p in range(4)]
    k_sb = [big.tile([P, T], F32, name=f"k_sb{p}") for p in range(4)]
    y_sb = [big.tile([P, T], F32, name=f"y_sb{p}") for p in range(4)]
    # V' = [V | 1] per head: [key%128, keychunk, head, 65]
    vp_sb = big.tile([P, T // P, H_PER_CORE, HS + 1], F32, name="vp_sb")
    nc.vector.memset(vp_sb[:, :, :, HS : HS + 1], 1.0)

    # ---- Phase 1: QKV projections (transposed Q/K, natural V) ----
    for nch in range(NCH):
        ts0 = nch * 512
        xt_tile = xin.tile([P, KCH, 512], F32, name=f"xt_{nch}", tag="xt")
        nc.sync.dma_start(out=xt_tile, in_=x_t3[:, :, ts0 : ts0 + 512])
        for mt in range(8):  # 4 Q pairs then 4 K pairs
            wt = wqk_p.tile([P, KCH, P], F32, name=f"wqk_{nch}_{mt}", tag="wqk")
            nc.sync.dma_start(out=wt, in_=w_qk3[:, :, mt * P : (mt + 1) * P])
            ps = psum.tile([P, 512], F32, name=f"p1_{nch}_{mt}", tag="p1")
            for kc in range(KCH):
                nc.tensor.matmul(
                    ps,
                    wt[:, kc, :],
                    xt_tile[:, kc, :],
                    start=(kc == 0),
                    stop=(kc == KCH - 1),
                )
            dst = q_sb[mt] if mt < 4 else k_sb[mt - 4]
            nc.any.tensor_copy(out=dst[:, ts0 : ts0 + 512], in_=ps)
        for tt in range(4):  # V for token chunks of 128
            ps = psum.tile([P, 512], F32, name=f"pv_{nch}_{tt}", tag="p1")
            for kc in range(KCH):
                nc.tensor.matmul(
                    ps,
                    xt_tile[:, kc, tt * P : (tt + 1) * P],
                    w_v_sb[:, kc, :],
                    start=(kc == 0),
                    stop=(kc == KCH - 1),
                )
            kchunk = nch * 4 + tt
            nc.any.tensor_copy(
                out=vp_sb[:, kchunk, :, 0:HS],
                in_=ps.rearrange("p (h d) -> p h d", h=H_PER_CORE),
            )

    # ---- Phase 2: causal attention per (qtile, head) ----
    for qt in range(QT):
        q0 = qt * 512
        nkc = 4 * (qt + 1)
        for h in range(H_PER_CORE):
            p, r0 = h // 2, (h % 2) * HS
            o_ps = psum.tile([HS + 1, 512], F32, name=f"o_{qt}_{h}", tag="o")
            for kc in range(nkc):
                c = kc - 4 * qt
                qoff = max(0, c) * P
                s_ps = psum.tile([P, 512], F32, name=f"s_{qt}_{h}_{kc}", tag="s")
                nc.tensor.matmul(
                    s_ps[:, qoff:],
                    k_sb[p][r0 : r0 + HS, kc * P : (kc + 1) * P],
                    q_sb[p][r0 : r0 + HS, q0 + qoff : q0 + 512],
                    start=True,
                    stop=True,
                )
                if c >= 0:
                    nc.vector.tensor_add(
                        out=s_ps[:, qoff : qoff + P],
                        in0=s_ps[:, qoff : qoff + P],
                        in1=mask_sb,
                    )
                exps = expp.tile([P, 512], F32, name=f"e_{qt}_{h}_{kc}", tag="exps")
                nc.scalar.activation(
                    out=exps[:, qoff:], in_=s_ps[:, qoff:], func=Exp, scale=SCALE
                )
                nc.tensor.matmul(
                    o_ps[:, qoff:],
                    vp_sb[:, kc, h, :],
                    exps[:, qoff:],
                    start=(kc == 0),
                    stop=(kc == nkc - 1),
                )
            recip = smallp.tile([1, 512], F32, name=f"r_{qt}_{h}", tag="recip")
            nc.vector.reciprocal(out=recip, in_=o_ps[HS : HS + 1, :])
            # Broadcast 1/l across 64 partitions: SBUF has no zero-step
            # partition reads, so bounce through DRAM (DRAM APs allow it).
            rd = dramp.tile([1, 512], F32, name=f"rd_{qt}_{h}", tag="rd")
            nc.sync.dma_start(out=rd, in_=recip)
            bc = smallp.tile([HS, 512], F32, name=f"bc_{qt}_{h}", tag="bc")
            nc.sync.dma_start(out=bc, in_=rd.partition_broadcast(HS))
            nc.vector.tensor_mul(
                out=y_sb[p][r0 : r0 + HS, q0 : q0 + 512],
                in0=o_ps[0:HS, :],
                in1=bc,
            )

    # ---- Phase 3: partial output projection (transposed) ----
    for nch in range(NCH):
        ts0 = nch * 512
        for mt in range(8):
            ps = psum.tile([P, 512], F32, name=f"po_{nch}_{mt}", tag="po")
            for kc in range(4):
                nc.tensor.matmul(
                    ps,
                    w_o_sb[:, kc, mt * P : (mt + 1) * P],
                    y_sb[kc][:, ts0 : ts0 + 512],
                    start=(kc == 0),
                    stop=(kc == 3),
                )
            ot = outsb.tile([P, 512], F32, name=f"ot_{nch}_{mt}", tag="ot")
            nc.any.tensor_copy(out=ot, in_=ps)
            nc.sync.dma_start(
                out=out2[mt * P : (mt + 1) * P, ts0 : ts0 + 512], in_=ot
            )


def build_nc(loop_k=None):
    """Build the per-core Bass module. loop_k wraps the body in a timing loop."""
    from contextlib import ExitStack

    nc = bass.Bass("TRN2")
    x_t = nc.dram_tensor("x_t", [C, T], F32, kind="ExternalInput")
    w_qk = nc.dram_tensor("w_qk", [C, 1024], F32, kind="ExternalInput")
    w_v = nc.dram_tensor("w_v", [C, 512], F32, kind="ExternalInput")
    w_o = nc.dram_tensor("w_o", [512, C], F32, kind="ExternalInput")
    mask = nc.dram_tensor("mask", [P, P], F32, kind="ExternalInput")
    out_t = nc.dram_tensor("out_t", [C, T], F32, kind="ExternalOutput")
    tensors = (x_t, w_qk, w_v, w_o, mask, out_t)

    with tile.TileContext(nc) as tc:
        with ExitStack() as ctx:
            if loop_k is None:
                build_attention_body(nc, tc, ctx, tensors)
            else:
                with tc.For_i(0, loop_k, 1):
                    build_attention_body(nc, tc, ctx, tensors)
    legalize_sync_waits(nc)
    return nc


def shard_inputs(x, w_attn, w_o):
    """Build the 8 per-core input maps."""
    x = np.asarray(x, dtype=np.float32)
    w_attn = np.asarray(w_attn, dtype=np.float32)
    w_o = np.asarray(w_o, dtype=np.float32)
    w_q, w_k, w_v = w_attn[:, 0:C], w_attn[:, C : 2 * C], w_attn[:, 2 * C : 3 * C]
    r = np.arange(P)
    mask = np.where(r[:, None] > r[None, :], np.float32(NEG), np.float32(0.0))
    in_maps = []
    for c in range(N_CORES):
        b, g = c // 2, c % 2
        heads = range(g * H_PER_CORE, (g + 1) * H_PER_CORE)
        w_qk_c = np.concatenate(
            [w_q[:, h * HS : (h + 1) * HS] for h in heads]
            + [w_k[:, h * HS : (h + 1) * HS] for h in heads],
            axis=1,
        )
        w_v_c = np.concatenate([w_v[:, h * HS : (h + 1) * HS] for h in heads], axis=1)
        in_maps.append(
            {
                "x_t": np.ascontiguousarray(x[b].T),
                "w_qk": np.ascontiguousarray(w_qk_c),
                "w_v": np.ascontiguousarray(w_v_c),
                "w_o": np.ascontiguousarray(w_o[g * 512 : (g + 1) * 512, :]),
                "mask": np.ascontiguousarray(mask, dtype=np.float32),
            }
        )
    return in_maps


def unshard_output(results):
    """Sum per-batch pair partials (the TP all-reduce) and untranspose."""
    out = np.empty((B, T, C), dtype=np.float32)
    for b in range(B):
        acc = results[2 * b]["out_t"] + results[2 * b + 1]["out_t"]
        out[b] = acc.T
    return out


# ---------------------------------------------------------------------------
# PJRT SPMD execution (axon): jit a shard_map over the 8 cores.
# ---------------------------------------------------------------------------


class SpmdRunner:
    def __init__(self, nc, n_cores=N_CORES):
        import jax
        from jax.sharding import Mesh, PartitionSpec
        from jax.experimental.shard_map import shard_map
        from concourse.bass2jax import (
            _bass_exec_p,
            install_neuronx_cc_hook,
            partition_id_tensor,
        )

        install_neuronx_cc_hook()
        self.jax = jax
        self.n_cores = n_cores
        partition_name = nc.partition_id_tensor.name if nc.partition_id_tensor else None
        in_names, out_names, out_avals, zero_outs = [], [], [], []
        for alloc in nc.m.functions[0].allocations:
            if not isinstance(alloc, mybir.MemoryLocationSet):
                continue
            name = alloc.memorylocations[0].name
            if alloc.kind == "ExternalInput":
                if name != partition_name:
                    in_names.append(name)
            elif alloc.kind == "ExternalOutput":
                out_names.append(name)
                shape = tuple(alloc.tensor_shape)
                dtype = mybir.dt.np(alloc.dtype)
                out_avals.append(jax.core.ShapedArray(shape, dtype))
                zero_outs.append(np.zeros(shape, dtype))
        self.in_names, self.out_names = in_names, out_names
        self.out_avals, self.zero_outs = out_avals, zero_outs
        n_params, n_outs = len(in_names), len(out_avals)
        all_in_names = in_names + out_names
        if partition_name is not None:
            all_in_names.append(partition_name)

        def _body(*args):
            operands = list(args)
            if partition_name is not None:
                operands.append(partition_id_tensor())
            return tuple(
                _bass_exec_p.bind(
                    *operands,
                    out_avals=tuple(out_avals),
                    in_names=tuple(all_in_names),
                    out_names=tuple(out_names),
                    lowering_input_output_aliases=(),
                    sim_require_finite=True,
                    sim_require_nnan=True,
                    nc=nc,
                )
            )

        devices = jax.devices()[:n_cores]
        assert len(devices) == n_cores, f"need {n_cores} cores, saw {jax.devices()}"
        self.mesh = Mesh(np.asarray(devices), ("core",))
        self.pspec = PartitionSpec("core")
        in_specs = (self.pspec,) * (n_params + n_outs)
        out_specs = (self.pspec,) * len(out_names)
        self.sharded = jax.jit(
            shard_map(
                _body,
                mesh=self.mesh,
                in_specs=in_specs,
                out_specs=out_specs,
                check_rep=False,
            ),
            keep_unused=True,
        )
        self.n_params = n_params

    def prepare(self, in_maps):
        from jax.sharding import NamedSharding

        per_core = [[np.asarray(m[n]) for n in self.in_names] for m in in_maps]
        concat_in = [
            np.concatenate([per_core[c][i] for c in range(self.n_cores)], axis=0)
            for i in range(self.n_params)
        ]
        concat_zeros = [
            np.zeros((self.n_cores * z.shape[0], *z.shape[1:]), z.dtype)
            for z in self.zero_outs
        ]
        sharding = NamedSharding(self.mesh, self.pspec)
        args = [self.jax.device_put(a, sharding) for a in concat_in + concat_zeros]
        self.jax.block_until_ready(args)
        return args

    def run(self, args):
        out = self.sharded(*args)
        self.jax.block_until_ready(out)
        return out

    def results(self, out_arrs):
        return [
            {
                name: np.asarray(out_arrs[i]).reshape(
                    self.n_cores, *self.out_avals[i].shape
                )[c]
                for i, name in enumerate(self.out_names)
            }
            for c in range(self.n_cores)
        ]


_RUNNER = None


def _get_runner():
    global _RUNNER
    if _RUNNER is None:
        _RUNNER = SpmdRunner(build_nc())
    return _RUNNER


def kernel(x, w_attn, w_o):
    runner = _get_runner()
    in_maps = shard_inputs(x, w_attn, w_o)
    args = runner.prepare(in_maps)
    out = runner.run(args)
    return unshard_output(runner.results(out))
